# revision 20
# baseline (speedup 1.0000x reference)
"""Trainium2 Bass kernel for nn_CustomLoss_38096359916115.

Computes, over y, y_echo, f of shape (8192, 4096) and scalar mu in (0,1):
  pseudo_l0_loss = sum((|y| >= mu) + (0 < |y| < mu) * |y| / mu) / n
                 = sum(min(|y|, mu)) / (mu * n)          (exact identity)
  l2_loss        = sum((y_echo / 22.8 - f)^2) / n

Strategy (data-parallel, memory-bound):
  - Shard the n=8192 row dimension across 8 NeuronCores (1024 rows each).
  - Per core, stream 8 blocks of [128, 4096] f32 per tensor from HBM
    (2 MiB contiguous HWDGE DMAs, double-buffered).
    L0 path: DVE tensor_scalar clamp(y, -mu, mu) in place, then ScalarE
             Abs activation with fused accum_out  -> sum(min(|y|, mu)).
    L2 path: DVE scalar_tensor_tensor (y_echo * (1/22.8)) - f in place,
             then ScalarE Square activation with fused accum_out.
  - Each block's two partial sums land in columns of a [128, 16] SBUF
    accumulator; one tiny DMA returns it per core.
  - Host sums the 8 x [128, 16] partials in float64 and applies the
    1/(mu*n) and 1/n scalings.

The program is traced under TileContext on a bacc.Bacc and then
nc.compile()d: the generate_event_semaphores pass splits multi-wait
instructions (TRN2 allows one sync wait per instruction) and
codegen_inst_isa_subclasses produces valid ISA encodings.
"""

import numpy as np

_ECHO_SCALE = 22.8
_P = 128
_N, _M = 8192, 4096
_NCORES = 8
_ROWS = _N // _NCORES  # rows per core
_NT = _ROWS // _P      # [128, _M] blocks per core

_cache = {}


def _ensure_path():
    try:
        import concourse  # noqa: F401
    except ImportError:
        import sys

        for p in ("/opt/trn_rl_repo", "/opt/pypackages"):
            if p not in sys.path:
                sys.path.append(p)
        import concourse  # noqa: F401


_CHUNK = 1024  # accum fold length; shorter folds -> smaller fp32 sum error


def build(rows=_ROWS, cols=_M, n_cores=_NCORES, bufs=3):
    """Trace + compile the per-core program. Returns (nc, n_tiles)."""
    _ensure_path()
    import concourse.mybir as mybir
    import concourse.tile as tile
    from concourse import bacc

    f32 = mybir.dt.float32
    Alu = mybir.AluOpType
    Act = mybir.ActivationFunctionType
    nt = rows // _P
    nch = max(1, cols // _CHUNK)  # accum chunks per tile
    chunk = cols // nch
    half = nt * nch  # accumulator columns per loss

    nc = bacc.Bacc(
        "TRN2", target_bir_lowering=False, debug=False, num_devices=n_cores
    )
    y = nc.dram_tensor("y", [rows, cols], f32, kind="ExternalInput").ap()
    ye = nc.dram_tensor("y_echo", [rows, cols], f32, kind="ExternalInput").ap()
    ff = nc.dram_tensor("f", [rows, cols], f32, kind="ExternalInput").ap()
    # column 0: mu, column 1: -mu
    mu2_b = nc.dram_tensor("mu2_b", [_P, 2], f32, kind="ExternalInput").ap()
    out = nc.dram_tensor("partials", [_P, 2 * half], f32, kind="ExternalOutput").ap()

    yt = y.rearrange("(n p) m -> n p m", p=_P)
    yet = ye.rearrange("(n p) m -> n p m", p=_P)
    fft = ff.rearrange("(n p) m -> n p m", p=_P)

    with tile.TileContext(nc) as tc:
        with (
            tc.tile_pool(name="consts", bufs=1) as cpool,
            tc.tile_pool(name="ldy", bufs=bufs) as ypool,
            tc.tile_pool(name="lde", bufs=bufs) as epool,
            tc.tile_pool(name="ldf", bufs=bufs) as fpool,
        ):
            mu_t = cpool.tile([_P, 2], f32)
            nc.sync.dma_start(mu_t[:], mu2_b[:])
            acc = cpool.tile([_P, 2 * half], f32)

            for i in range(nt):
                ty = ypool.tile([_P, cols], f32)
                nc.sync.dma_start(ty[:], yt[i])
                te = epool.tile([_P, cols], f32)
                nc.sync.dma_start(te[:], yet[i])
                tf = fpool.tile([_P, cols], f32)
                nc.sync.dma_start(tf[:], fft[i])

                # ty = clamp(y, -mu, mu); |clamp| == min(|y|, mu)
                nc.vector.tensor_scalar(
                    ty[:], ty[:], mu_t[:, 0:1], mu_t[:, 1:2], Alu.min, Alu.max
                )
                # te = (y_echo * (1/22.8)) - f
                nc.vector.scalar_tensor_tensor(
                    te[:], te[:], 1.0 / _ECHO_SCALE, tf[:], Alu.mult, Alu.subtract
                )
                # accum per chunk: sum(|clamp|) and sum(diff^2) along free dim
                for j in range(nch):
                    cs = slice(j * chunk, (j + 1) * chunk)
                    col = i * nch + j
                    nc.scalar.activation(
                        ty[:, cs],
                        ty[:, cs],
                        Act.Abs,
                        accum_out=acc[:, col : col + 1],
                    )
                    nc.scalar.activation(
                        te[:, cs],
                        te[:, cs],
                        Act.Square,
                        accum_out=acc[:, half + col : half + col + 1],
                    )

            nc.sync.dma_start(out[:], acc[:])

    nc.compile()
    return nc, nt


def _get_nc():
    if "nc" not in _cache:
        _cache["nc"] = build()
    return _cache["nc"]


def make_in_maps(y, y_echo, f, mu, rows=_ROWS, n_cores=_NCORES):
    mu_f = float(np.asarray(mu).reshape(-1)[0])
    mu2 = np.empty((_P, 2), np.float32)
    mu2[:, 0] = mu_f
    mu2[:, 1] = -mu_f
    in_maps = []
    for c in range(n_cores):
        sl = slice(c * rows, (c + 1) * rows)
        in_maps.append(
            {
                "y": np.ascontiguousarray(y[sl]),
                "y_echo": np.ascontiguousarray(y_echo[sl]),
                "f": np.ascontiguousarray(f[sl]),
                "mu2_b": mu2,
            }
        )
    return in_maps


def reduce_partials(partials, mu):
    """partials: list/array of per-core [128, 2*half] f32 -> (l0, l2) f32."""
    mu_f = float(np.asarray(mu).reshape(-1)[0])
    parts = np.asarray(partials, dtype=np.float64)
    half = parts.shape[-1] // 2
    s0 = parts[..., :half].sum()
    s1 = parts[..., half:].sum()
    l0 = np.float32(s0 / (mu_f * _N))
    l2 = np.float32(s1 / _N)
    return np.asarray(l0, np.float32), np.asarray(l2, np.float32)


def kernel(y, y_echo, f, mu):
    _ensure_path()
    from concourse.bass_utils import run_bass_kernel_spmd

    nc, nt = _get_nc()
    in_maps = make_in_maps(y, y_echo, f, mu)
    res = run_bass_kernel_spmd(nc, in_maps, list(range(_NCORES))).results
    partials = [r["partials"] for r in res]
    return reduce_partials(partials, mu)
